# revision 1
# baseline (speedup 1.0000x reference)
"""Trainium2 Bass kernel for nn_AttentionBlock (GroupNorm + 1x1 conv QKV + MHA + out-proj + residual).

Sharding: 8 cores = 2 batches x 4 heads. Each core computes GroupNorm for its
batch (duplicated), the qkv projection rows for its head, full [4096 x 4096]
attention for its (batch, head), and the partial output projection
w_out[:, head] @ a (unnormalized by the softmax denominator Z). The host
divides by Z, sums the 4 head partials per batch, and adds b_out + residual.

Device math notes:
  - softmax computed without max-subtraction (scores are bounded ~|7|, exp is
    safe in fp32); scale 1/sqrt(sqrt(ch)) folded into the q/k weights on host.
  - scores are computed directly in [s, t] orientation (S2 = k^T q) so the
    softmax contraction dim s lands on partitions for the AV matmul.
  - Z obtained for free via a ones-column appended to v^T (65th matmul column).
  - GroupNorm group stats via per-channel bn_stats + group-mask matmul.
"""

import os
import sys

import numpy as np

if os.path.isdir("/opt/trn_rl_repo") and "/opt/trn_rl_repo" not in sys.path:
    sys.path.insert(0, "/opt/trn_rl_repo")

import concourse.bass as bass
import concourse.mybir as mybir
import concourse.tile as tile
from concourse import bacc
from concourse.bass import ts

P = 128
L = 4096          # D*H*W
T = 512           # t-chunk size
NCHUNK = L // T   # 8
NST = L // P      # 32 s-tiles
CH = 64           # head dim
EPS = 1e-6
F32 = mybir.dt.float32
F32R = mybir.dt.float32r
N_CORES = 8


def build_attention_nc():
    """Build the single-core SPMD Bass program."""
    from contextlib import ExitStack

    nc = bacc.Bacc("TRN2", target_bir_lowering=False, debug=False, num_devices=N_CORES)
    AF = mybir.ActivationFunctionType
    OP = mybir.AluOpType

    xin = nc.dram_tensor("xin", [P, 2, L], F32, kind="ExternalInput").ap()
    wqkvT = nc.dram_tensor("wqkvT", [P, 2, 320], F32, kind="ExternalInput").ap()
    bqk_d = nc.dram_tensor("bqk", [P, 2], F32, kind="ExternalInput").ap()
    bv_d = nc.dram_tensor("bv", [CH], F32, kind="ExternalInput").ap()
    woutT = nc.dram_tensor("woutT", [CH, 2, P], F32, kind="ExternalInput").ap()
    gnsc_d = nc.dram_tensor("gnsc", [P, 2], F32, kind="ExternalInput").ap()
    gnbi_d = nc.dram_tensor("gnbi", [P, 2], F32, kind="ExternalInput").ap()
    gmask_d = nc.dram_tensor("gmask_in", [P, 8], F32, kind="ExternalInput").ap()
    yp_d = nc.dram_tensor("yp", [P, 2, L], F32, kind="ExternalOutput").ap()
    z_d = nc.dram_tensor("zout", [1, L], F32, kind="ExternalOutput").ap()

    with tile.TileContext(nc) as tc, ExitStack() as ctx:
        big = ctx.enter_context(tc.tile_pool(name="big", bufs=2))
        persist = ctx.enter_context(tc.tile_pool(name="persist", bufs=1))
        small = ctx.enter_context(tc.tile_pool(name="small", bufs=1))
        work = ctx.enter_context(tc.tile_pool(name="work", bufs=2))

        # ---- persistent tiles ----
        xt = big.tile([P, 2, L], F32, tag="big", name="xt")
        # qk2[:,0,:] = [q;k] (partitions 0:64 / 64:128), qk2[:,1,:] = [k;q]
        qk2 = persist.tile([P, 2, L], F32R, name="qk2")
        vt = persist.tile([P, NST, CH + 1], F32R, name="vt")  # v^T blocks + ones col
        wq_raw = persist.tile([P, 2, 320], F32, name="wq_raw")
        wq_sb = persist.tile([P, 2, 320], F32R, name="wq_sb")
        wo_raw = persist.tile([CH, 2, P], F32, name="wo_raw")
        wo_sb = persist.tile([CH, 2, P], F32R, name="wo_sb")
        gmask_raw = persist.tile([P, 8], F32, name="gmask_raw")
        bqk_sb = persist.tile([P, 2], F32, name="bqk_sb")
        bvbc = persist.tile([P, CH], F32, name="bvbc")
        gnsc_sb = persist.tile([P, 2], F32, name="gnsc_sb")
        gnbi_sb = persist.tile([P, 2], F32, name="gnbi_sb")
        gmask = persist.tile([P, 8], F32, name="gmask")

        # ---- input DMAs ----
        for po in range(2):
            for hh in range(2):
                nc.sync.dma_start(xt[:, po, ts(hh, L // 2)],
                                  xin[:, po, ts(hh, L // 2)])
        nc.sync.dma_start(wq_raw, wqkvT)
        nc.sync.dma_start(wo_raw, woutT)
        nc.vector.tensor_copy(wq_sb, wq_raw)
        nc.vector.tensor_copy(wo_sb, wo_raw)
        nc.sync.dma_start(bqk_sb, bqk_d)
        # broadcast bv [64] across all 128 partitions
        bv_bcast = bass.AP(tensor=bv_d.tensor, offset=bv_d.offset,
                           ap=[[0, P]] + list(bv_d.ap))
        nc.sync.dma_start(bvbc, bv_bcast)
        nc.sync.dma_start(gnsc_sb, gnsc_d)
        nc.sync.dma_start(gnbi_sb, gnbi_d)

        nc.sync.dma_start(gmask_raw, gmask_d)
        nc.vector.tensor_copy(gmask, gmask_raw)
        # ones column of vt (f32r memset is invalid ISA; use 0*x+1 instead)
        nc.vector.tensor_scalar(vt[:, :, CH:CH + 1],
                                bvbc[:, 0:NST].rearrange("p a -> p a ()"),
                                0.0, 1.0, OP.mult, OP.add)

        # ---- GroupNorm stats ----
        stats = small.tile([P, 2, 8, 6], F32, name="stats")
        mv = small.tile([P, 2, 2], F32, name="mv")
        for po in range(2):
            for i in range(8):
                nc.vector.bn_stats(stats[:, po, i, :], xt[:, po, ts(i, 512)])
            nc.vector.bn_aggr(mv[:, po, :], stats[:, po, :, :])
        rhs_gs = small.tile([P, 4], F32, name="rhs_gs")   # [m0 m1 s0 s1]
        nc.vector.tensor_copy(rhs_gs[:, 0:2], mv[:, :, 0])
        nc.vector.tensor_tensor(rhs_gs[:, 2:4], mv[:, :, 0], mv[:, :, 0], OP.mult)
        nc.vector.tensor_tensor(rhs_gs[:, 2:4], rhs_gs[:, 2:4], mv[:, :, 1], OP.add)

        with tc.tile_pool(name="pre_ps", bufs=2, space="PSUM") as pre_ps:
            # group sums: [8, 4] = gmask.T @ rhs_gs
            psg = pre_ps.tile([8, 4], F32, tag="g", name="psg")
            nc.tensor.matmul(psg, gmask, rhs_gs, start=True, stop=True)
            mg = small.tile([8, 2], F32, name="mg")
            varg = small.tile([8, 2], F32, name="varg")
            rstd = small.tile([8, 2], F32, name="rstd")
            tmp8 = small.tile([8, 2], F32, name="tmp8")
            epst = small.tile([8, 1], F32, name="epst")
            nc.vector.memset(epst, EPS)
            nc.vector.tensor_scalar_mul(mg, psg[:, 0:2], 1.0 / 16.0)
            nc.vector.tensor_scalar_mul(varg, psg[:, 2:4], 1.0 / 16.0)
            nc.vector.tensor_tensor(tmp8, mg, mg, OP.mult)
            nc.vector.tensor_tensor(varg, varg, tmp8, OP.subtract)
            nc.scalar.activation(varg, varg, AF.Sqrt, bias=epst)  # sqrt(var+eps)
            nc.vector.reciprocal(rstd, varg)
            # touch Exp now so the ACT table set loads during the pre-phase
            warmup = small.tile([8, 1], F32, name="warmup")
            nc.scalar.activation(warmup, epst, AF.Exp)

            # broadcast group stats to channels: [8,2] -> [128,2] (repeat 16x)
            rstdc = small.tile([P, 2], F32, name="rstdc")
            mgc = small.tile([P, 2], F32, name="mgc")
            for src_, dst in ((rstd, rstdc), (mg, mgc)):
                rep = bass.AP(tensor=src_.tensor, offset=src_.offset,
                              ap=[list(src_.ap[0]), [0, 16], list(src_.ap[1])])
                nc.sync.dma_start(dst, rep)
            a_aff = small.tile([P, 2], F32, name="a_aff")
            b_aff = small.tile([P, 2], F32, name="b_aff")
            tmpc = small.tile([P, 2], F32, name="tmpc")
            nc.vector.tensor_tensor(a_aff, rstdc, gnsc_sb, OP.mult)
            nc.vector.tensor_tensor(tmpc, mgc, a_aff, OP.mult)
            nc.vector.tensor_tensor(b_aff, gnbi_sb, tmpc, OP.subtract)
            # xn = x*A + B (f32r output for the matmuls)
            xn = big.tile([P, 2, L], F32R, tag="big", name="xn")
            for po in range(2):
                nc.vector.tensor_scalar(xn[:, po, :], xt[:, po, :],
                                        a_aff[:, po:po + 1], b_aff[:, po:po + 1],
                                        OP.mult, OP.add)

            # ---- qkv projection ----
            # v^T blocks first: vt[:, j, 0:64] = xn[:, jP:(j+1)P]^T @ Wv^T + bv
            for j in range(NST):
                ps_vt = pre_ps.tile([P, CH], F32, tag="vt", name="ps_vt")
                for ko in range(2):
                    nc.tensor.matmul(ps_vt, xn[:, ko, ts(j, P)], wq_sb[:, ko, 128:192],
                                     start=(ko == 0), stop=(ko == 1))
                nc.vector.tensor_tensor(vt[:, j, 0:CH], ps_vt, bvbc, OP.add)
            # qk: [q;k] and swapped [k;q] layouts; one [128,T] bias-copy each
            for ic in range(NCHUNK):
                ps_qk = pre_ps.tile([P, T], F32, tag="qk", name="ps_qk", bufs=4)
                ps_kq = pre_ps.tile([P, T], F32, tag="qk", name="ps_kq", bufs=4)
                for ko in range(2):
                    nc.tensor.matmul(ps_qk, wq_sb[:, ko, 0:128], xn[:, ko, ts(ic, T)],
                                     start=(ko == 0), stop=(ko == 1))
                for ko in range(2):
                    nc.tensor.matmul(ps_kq, wq_sb[:, ko, 192:320], xn[:, ko, ts(ic, T)],
                                     start=(ko == 0), stop=(ko == 1))
                nc.vector.tensor_scalar_add(qk2[:, 0, ts(ic, T)], ps_qk,
                                            bqk_sb[:, 0:1])
                nc.vector.tensor_scalar_add(qk2[:, 1, ts(ic, T)], ps_kq,
                                            bqk_sb[:, 1:2])

        # ---- attention ----
        # Per-chunk structure: S2 trickle (exp-paced) then a dense AV burst
        # (keeps PE's HAM warm). The first LA S2 groups of the NEXT chunk are
        # emitted interspersed inside the AV burst (after their PSUM slots
        # free) so ScalarE keeps doing exps during the burst.
        LA = 4
        with tc.tile_pool(name="att_s", bufs=2, space="PSUM") as sps, \
                tc.tile_pool(name="att_a", bufs=1, space="PSUM") as aps, \
                tc.tile_pool(name="att_y", bufs=1, space="PSUM") as yps:
            e2s = {}

            def emit_s2_group(ic, gstart, gsize):
                e2 = e2s[ic]
                ps_s = sps.tile([P, 3, T], F32, tag="s", name="ps_s")
                for jj in range(gsize):
                    sj = gstart + jj
                    hb = (sj % 2) * CH
                    kv = 1 - (sj % 2)
                    qv = sj % 2
                    nc.tensor.matmul(ps_s[:, jj, :],
                                     qk2[hb:hb + CH, kv, ts(sj, P)],
                                     qk2[hb:hb + CH, qv, ts(ic, T)],
                                     start=True, stop=True,
                                     tile_position=(hb, 0))
                nc.scalar.activation(e2[:, gstart:gstart + gsize, :],
                                     ps_s[:, 0:gsize, :], AF.Exp)

            groups = []
            g0 = 0
            while g0 < NST:
                groups.append((g0, min(3, NST - g0)))
                g0 += min(3, NST - g0)
            # AV positions at which to inject the next chunk's lookahead groups
            inject_at = {6: 0, 12: 1, 18: 2, 24: 3}

            for ic in range(NCHUNK):
                if ic == 0:
                    e2s[0] = big.tile([P, NST, T], F32R, tag="big", name="e2")
                    for gstart, gsize in groups[:LA]:
                        emit_s2_group(0, gstart, gsize)
                for gstart, gsize in groups[LA:]:
                    emit_s2_group(ic, gstart, gsize)
                if ic + 1 < NCHUNK:
                    e2s[ic + 1] = big.tile([P, NST, T], F32R, tag="big", name="e2")
                # AV burst with lookahead injections
                e2 = e2s.pop(ic)
                ps_a = aps.tile([P, T], F32, tag="a", name="ps_a")
                for j in range(NST):
                    nc.tensor.matmul(ps_a[0:CH + 1, :], vt[:, j, :], e2[:, j, :],
                                     start=(j == 0), stop=(j == NST - 1))
                    gi = inject_at.get(j)
                    if gi is not None and ic + 1 < NCHUNK:
                        emit_s2_group(ic + 1, *groups[gi])
                azt = work.tile([CH + 1, T], F32R, tag="az", name="azt")
                nc.vector.tensor_copy(azt, ps_a[0:CH + 1, :])
                zt = work.tile([CH + 1, T], F32, tag="zt", name="zt")
                nc.vector.tensor_copy(zt[CH:CH + 1, :], ps_a[CH:CH + 1, :])
                nc.sync.dma_start(z_d[0:1, ts(ic, T)], zt[CH:CH + 1, :])
                ysb = work.tile([P, 2, T], F32, tag="y", name="ysb")
                for mo in range(2):
                    ps_y = yps.tile([P, T], F32, tag="y", name="ps_y")
                    nc.tensor.matmul(ps_y, wo_sb[:, mo, :], azt[0:CH, :],
                                     start=True, stop=True)
                    nc.vector.tensor_copy(ysb[:, mo, :], ps_y)
                nc.sync.dma_start(yp_d[:, :, ts(ic, T)], ysb)

    nc.compile()
    return nc


def make_core_inputs(x, gn_scale, gn_bias, w_qkv, b_qkv, w_out, b_out):
    """Shard full inputs into 8 per-core input maps (batch n, head h)."""
    N, C, D, H, W = x.shape
    l = D * H * W
    xf = np.ascontiguousarray(x.reshape(N, C, l), dtype=np.float32)
    scale = np.float32(1.0 / np.sqrt(np.sqrt(CH)))
    gnsc = np.ascontiguousarray(gn_scale.reshape(2, P).T, dtype=np.float32)
    gnbi = np.ascontiguousarray(gn_bias.reshape(2, P).T, dtype=np.float32)
    in_maps = []
    for core in range(N_CORES):
        n, h = divmod(core, 4)
        xn_ = np.ascontiguousarray(
            xf[n].reshape(2, P, l).transpose(1, 0, 2))
        wq_h = w_qkv[h * CH:(h + 1) * CH] * scale
        wk_h = w_qkv[C + h * CH:C + (h + 1) * CH] * scale
        wv_h = w_qkv[2 * C + h * CH:2 * C + (h + 1) * CH]
        rows = np.concatenate([wq_h, wk_h, wv_h, wk_h, wq_h], axis=0)  # [320, 256]
        wq = np.ascontiguousarray(
            rows.T.reshape(2, P, 320).transpose(1, 0, 2), dtype=np.float32)
        bq_h = b_qkv[h * CH:(h + 1) * CH] * scale
        bk_h = b_qkv[C + h * CH:C + (h + 1) * CH] * scale
        bqk = np.stack([np.concatenate([bq_h, bk_h]),
                        np.concatenate([bk_h, bq_h])], axis=1).astype(np.float32)
        bv = np.ascontiguousarray(b_qkv[2 * C + h * CH:2 * C + (h + 1) * CH],
                                  dtype=np.float32)
        wo = np.ascontiguousarray(
            w_out[:, h * CH:(h + 1) * CH].T.reshape(CH, 2, P), dtype=np.float32)
        gm = np.zeros((P, 8), np.float32)
        for g in range(8):
            gm[g * 16:(g + 1) * 16, g] = 1.0
        in_maps.append({
            "xin": xn_, "wqkvT": wq, "bqk": np.ascontiguousarray(bqk),
            "bv": bv, "woutT": wo, "gnsc": gnsc, "gnbi": gnbi, "gmask_in": gm,
        })
    return in_maps


def combine_outputs(results, x, b_out):
    """Host gather: y = sum_h yp/z per batch + b_out + residual."""
    N, C, D, H, W = x.shape
    l = D * H * W
    xf = x.reshape(N, C, l)
    y = np.zeros((N, C, l), np.float32)
    for core, res in enumerate(results):
        n = core // 4
        yp = res["yp"].reshape(P, 2, l).transpose(1, 0, 2).reshape(C, l)
        z = res["zout"].reshape(l)
        y[n] += yp / z[None, :]
    y += b_out.astype(np.float32)[None, :, None] + xf
    return y.reshape(N, C, D, H, W).astype(np.float32)


_NC_CACHE = {}


def get_nc():
    if "nc" not in _NC_CACHE:
        _NC_CACHE["nc"] = build_attention_nc()
    return _NC_CACHE["nc"]


def kernel(x, gn_scale, gn_bias, w_qkv, b_qkv, w_out, b_out, _trace=False):
    from concourse.bass_utils import run_bass_kernel_spmd
    x = np.asarray(x); gn_scale = np.asarray(gn_scale); gn_bias = np.asarray(gn_bias)
    w_qkv = np.asarray(w_qkv); b_qkv = np.asarray(b_qkv)
    w_out = np.asarray(w_out); b_out = np.asarray(b_out)
    nc = get_nc()
    in_maps = make_core_inputs(x, gn_scale, gn_bias, w_qkv, b_qkv, w_out, b_out)
    res = run_bass_kernel_spmd(nc, in_maps, core_ids=list(range(N_CORES)),
                               trace=_trace)
    out = combine_outputs(res.results, x, b_out)
    if _trace:
        kernel.last_results = res
    return out


if __name__ == "__main__":
    sys.path.insert(0, os.path.dirname(os.path.abspath(__file__)))
    import reference
    inputs = {k: np.asarray(v) for k, v in reference.setup_inputs().items()}
    expected = np.asarray(reference.reference(**inputs))
    got = kernel(**inputs)
    err = np.abs(got - expected).max()
    rel = err / np.abs(expected).max()
    print("abs err:", err, "rel err:", rel)



# revision 2
# speedup vs baseline: 1.0247x; 1.0247x over previous
"""Trainium2 Bass kernel v2 for nn_AttentionBlock (GroupNorm + QKV + MHA + out-proj).

Sharding: 8 cores = 2 batches x 4 heads (as baseline). Each core: GroupNorm
for its batch, its head's qkv projection, full [4096 x 4096] attention, and
the partial (unnormalized) out-projection. Host divides by Z, sums heads,
adds b_out + residual.

v2 redesign vs baseline:
  - exp split across TWO engines: ACT computes exp->fp8e4 directly for the
    even groups; DVE computes a Schraudolph fast-exp (f32 -> int16 write into
    a bf16 tile == bf16(exp)) for the odd groups. This nearly halves the
    serial exp cost, which dominated the baseline.
  - AV matmuls consume fp8 tiles via DoubleRow perf mode (2 s-tiles per
    matmul) and bf16 tiles at 1 cyc/row; qkv/S2/yproj in bf16.
  - AV for chunk ic is interleaved into chunk ic's own S2/exp stream with a
    2-group lag (no big serial AV burst; short tail).
  - Head phase (GN stats + affine + qkv projection) shares the S2 PSUM slot
    rotation and is interleaved into chunk 0's stream.
  - y partials and Z exported in bf16 (halves output DMA).
"""

import os
import sys

import numpy as np
import ml_dtypes

if os.path.isdir("/opt/trn_rl_repo") and "/opt/trn_rl_repo" not in sys.path:
    sys.path.insert(0, "/opt/trn_rl_repo")

import concourse.bass as bass
import concourse.mybir as mybir
import concourse.tile as tile
from concourse import bacc
from concourse.bass import ts

P = 128
L = 4096          # D*H*W
T = 512           # t-chunk size
NCHUNK = L // T   # 8
NST = L // P      # 32 s-tiles
CH = 64           # head dim
EPS = 1e-6
F32 = mybir.dt.float32
BF16 = mybir.dt.bfloat16
U8 = mybir.dt.uint8
FP8 = mybir.dt.float8e4
N_CORES = 8

# exp(s - CSHIFT) keeps e2 in fp8e4(max 240, overflow -> inf!) for s <= ~8.0;
# scores are ~N(0,1) so P(any of 134M samples > 8) ~ 0
CSHIFT = 2.5
# uint8 Schraudolph: uint8(round(s*K8 + C8)) bit-pattern == fp8e4(exp(s-CSHIFT));
# negative values saturate to 0 on the f32->uint8 convert (== dropped weight)
K8 = 8.0 / np.log(2.0)
C8 = 56.0 - 0.0428 * 8.0 - CSHIFT * K8

# groups of 2 s-tiles per chunk -> 16 groups; each group == one DR pair
NG = NST // 2
# engine per group: 'A' = ACT exp->fp8, 'D' = DVE uint8-schraudolph->fp8 (9A/7D)
PATTERN = ['A', 'D', 'A', 'D', 'A', 'D', 'A', 'A',
           'D', 'A', 'D', 'A', 'D', 'A', 'D', 'A']
LAG = 3


def build_attention_nc():
    from contextlib import ExitStack

    nc = bacc.Bacc("TRN2", target_bir_lowering=False, debug=False, num_devices=N_CORES)
    AF = mybir.ActivationFunctionType
    OP = mybir.AluOpType
    DR = mybir.MatmulPerfMode.DoubleRow

    # chunk-major input: [P, chunk, po, T] -> 2KB contiguous runs per
    # partition per chunk (DMA issue cost is per-descriptor)
    xin = nc.dram_tensor("xin", [P, NCHUNK, 2, T], BF16, kind="ExternalInput").ap()
    wqkvT = nc.dram_tensor("wqkvT", [P, 2, 320], F32, kind="ExternalInput").ap()
    bqk_d = nc.dram_tensor("bqk", [P, 2], F32, kind="ExternalInput").ap()
    bv_d = nc.dram_tensor("bv", [CH], F32, kind="ExternalInput").ap()
    woutT = nc.dram_tensor("woutT", [CH, 2, P], F32, kind="ExternalInput").ap()
    gnsc_d = nc.dram_tensor("gnsc", [P, 2], F32, kind="ExternalInput").ap()
    gnbi_d = nc.dram_tensor("gnbi", [P, 2], F32, kind="ExternalInput").ap()
    gmask_d = nc.dram_tensor("gmask_in", [P, 8], F32, kind="ExternalInput").ap()
    yp_d = nc.dram_tensor("yp", [P, 2, L], BF16, kind="ExternalOutput").ap()
    z_d = nc.dram_tensor("zout", [1, L], BF16, kind="ExternalOutput").ap()

    with tile.TileContext(nc) as tc, ExitStack() as ctx:
        big = ctx.enter_context(tc.tile_pool(name="big", bufs=1))
        e2p = ctx.enter_context(tc.tile_pool(name="e2p", bufs=2))
        small = ctx.enter_context(tc.tile_pool(name="small", bufs=1))
        work = ctx.enter_context(tc.tile_pool(name="work", bufs=2))
        pss = ctx.enter_context(tc.tile_pool(name="pss", bufs=3, space="PSUM"))
        psa = ctx.enter_context(tc.tile_pool(name="psa", bufs=1, space="PSUM"))
        psy = ctx.enter_context(tc.tile_pool(name="psy", bufs=1, space="PSUM"))

        # ---- persistent tiles ----
        xt = big.tile([P, NCHUNK, 2, T], BF16, name="xt")
        xn = big.tile([P, NCHUNK, 2, T], BF16, name="xn")
        qk2 = big.tile([P, 2, L], BF16, name="qk2")
        vt = big.tile([P, NST, CH + 1], BF16, name="vt")
        # fp8 copy padded to 80 cols: DoubleRow ldweights needs the outer free
        # stride even and 16B-aligned
        vt8 = big.tile([P, NST, 80], FP8, name="vt8")
        wq_raw = big.tile([P, 2, 320], F32, name="wq_raw")
        wq_sb = big.tile([P, 2, 320], BF16, name="wq_sb")
        wo_raw = big.tile([CH, 2, P], F32, name="wo_raw")
        wo_sb = big.tile([CH, 2, P], BF16, name="wo_sb")
        gmask = big.tile([P, 8], F32, name="gmask")
        bqk_sb = big.tile([P, 2], F32, name="bqk_sb")
        bvbc = big.tile([P, 4, CH], F32, name="bvbc")
        gnsc_sb = big.tile([P, 2], F32, name="gnsc_sb")
        gnbi_sb = big.tile([P, 2], F32, name="gnbi_sb")
        bneg = big.tile([P, 1], F32, name="bneg")

        # ---- input DMA: one joint-po DMA per x-chunk (DMA-queue issue costs
        # ~600ns per descriptor, so fewer+bigger). Small weight DMAs go after
        # chunk 3 so they arrive well before the projection needs them. ----
        nc.vector.memset(bneg, -CSHIFT)
        # warm the Exp table set asap
        warm = small.tile([P, 1], F32, name="warm")
        nc.scalar.activation(warm, bneg, AF.Exp)

        stats = small.tile([P, 2, 8, 6], F32, name="stats")

        # one DMA per chunk-major x chunk (2KB runs -> few descriptors), all on
        # the sync queue; the ONE dma engine serializes transfers anyway, so
        # multi-queue only helps issue, not completion. Weights after.
        for i in range(8):
            nc.sync.dma_start(xt[:, i], xin[:, i])
        nc.sync.dma_start(gmask, gmask_d)
        nc.sync.dma_start(gnsc_sb, gnsc_d)
        nc.sync.dma_start(gnbi_sb, gnbi_d)
        nc.sync.dma_start(wq_raw, wqkvT)
        nc.sync.dma_start(bqk_sb, bqk_d)
        bv_bcast = bass.AP(tensor=bv_d.tensor, offset=bv_d.offset,
                           ap=[[0, P], [0, 4]] + list(bv_d.ap))
        nc.sync.dma_start(bvbc, bv_bcast)
        nc.sync.dma_start(wo_raw, woutT)
        # per-chunk bn_stats (bn_stats free size is hw-capped at 512)
        for i in range(8):
            for po in range(2):
                nc.vector.bn_stats(stats[:, po, i, :], xt[:, i, po, :])
        nc.vector.tensor_copy(wq_sb, wq_raw)
        nc.vector.tensor_copy(wo_sb, wo_raw)
        # ones columns of vt / vt8; zero the vt8 padding
        nc.vector.memset(vt[:, :, CH:CH + 1], 1.0)
        nc.vector.memset(vt8[:, :, CH:], 0.0)
        nc.vector.memset(vt8[:, :, CH:CH + 1], 1.0)
        mv = small.tile([P, 2, 2], F32, name="mv")
        for po in range(2):
            nc.vector.bn_aggr(mv[:, po, :], stats[:, po, :, :])
        rhs_gs = small.tile([P, 4], F32, name="rhs_gs")   # [m0 m1 s0 s1]
        nc.vector.tensor_copy(rhs_gs[:, 0:2], mv[:, :, 0])
        nc.vector.tensor_tensor(rhs_gs[:, 2:4], mv[:, :, 0], mv[:, :, 0], OP.mult)
        nc.vector.tensor_tensor(rhs_gs[:, 2:4], rhs_gs[:, 2:4], mv[:, :, 1], OP.add)

        psg = pss.tile([8, 4], F32, tag="s", name="psg")
        nc.tensor.matmul(psg, gmask, rhs_gs, start=True, stop=True)
        rm = small.tile([8, 4], F32, name="rm")   # [rstd0 rstd1 mg0 mg1]
        varg = small.tile([8, 2], F32, name="varg")
        tmp8 = small.tile([8, 2], F32, name="tmp8")
        epst = small.tile([8, 1], F32, name="epst")
        nc.vector.memset(epst, EPS)
        nc.vector.tensor_scalar_mul(rm[:, 2:4], psg[:, 0:2], 1.0 / 16.0)
        nc.vector.tensor_scalar_mul(varg, psg[:, 2:4], 1.0 / 16.0)
        nc.vector.tensor_tensor(tmp8, rm[:, 2:4], rm[:, 2:4], OP.mult)
        nc.vector.tensor_tensor(varg, varg, tmp8, OP.subtract)
        nc.scalar.activation(varg, varg, AF.Sqrt, bias=epst)
        nc.vector.reciprocal(rm[:, 0:2], varg)

        rmc = small.tile([P, 4], F32, name="rmc")
        rep = bass.AP(tensor=rm.tensor, offset=rm.offset,
                      ap=[list(rm.ap[0]), [0, 16], list(rm.ap[1])])
        nc.sync.dma_start(rmc, rep)
        a_aff = small.tile([P, 2], F32, name="a_aff")
        b_aff = small.tile([P, 2], F32, name="b_aff")
        tmpc = small.tile([P, 2], F32, name="tmpc")
        nc.vector.tensor_tensor(a_aff, rmc[:, 0:2], gnsc_sb, OP.mult)
        nc.vector.tensor_tensor(tmpc, rmc[:, 2:4], a_aff, OP.mult)
        nc.vector.tensor_tensor(b_aff, gnbi_sb, tmpc, OP.subtract)

        # ---------------- pipelined head + attention ----------------
        def emit_xn(h):
            # bf16-in/bf16-out SBUF: DVE runs these in 2x/4x perf mode
            for po in range(2):
                nc.vector.tensor_scalar(xn[:, h, po, :], xt[:, h, po, :],
                                        a_aff[:, po:po + 1], b_aff[:, po:po + 1],
                                        OP.mult, OP.add)

        def emit_qkkq(h):
            # both variants in one 2-bank PSUM unit; drains split DVE/ACT
            ps_qk = pss.tile([P, 2, T], F32, tag="s", name="ps_qk")
            for var in range(2):
                c0 = 0 if var == 0 else 192
                for ko in range(2):
                    nc.tensor.matmul(ps_qk[:, var, :], wq_sb[:, ko, c0:c0 + 128],
                                     xn[:, h, ko, :],
                                     start=(ko == 0), stop=(ko == 1))
            nc.vector.tensor_scalar_add(qk2[:, 0, ts(h, T)], ps_qk[:, 0, :],
                                        bqk_sb[:, 0:1])
            nc.scalar.activation(qk2[:, 1, ts(h, T)], ps_qk[:, 1, :], AF.Identity,
                                 bias=bqk_sb[:, 1:2])

        def emit_vt(h):
            # 4 s-tiles of v^T for x-chunk h, one 1-bank PSUM unit on "y"
            ps_vt = psy.tile([P, 4, CH], F32, tag="y", name="ps_vt")
            for jj in range(4):
                for ko in range(2):
                    nc.tensor.matmul(ps_vt[:, jj, :],
                                     xn[:, h, ko, ts(jj, P)],
                                     wq_sb[:, ko, 128:192],
                                     start=(ko == 0), stop=(ko == 1))
            nc.vector.tensor_tensor(vt[:, 4 * h:4 * h + 4, 0:CH], ps_vt, bvbc,
                                    OP.add)
            nc.scalar.copy(vt8[:, 4 * h:4 * h + 4, 0:CH],
                           vt[:, 4 * h:4 * h + 4, 0:CH])

        head_units = []
        for h in range(NCHUNK):
            head_units.append((h, 'xn'))
            head_units.append((h, 'qkkq'))
            head_units.append((h, 'vt'))
        head_idx = 0

        def run_head_until(h_done):
            """Emit head units until chunks 0..h_done are fully emitted."""
            nonlocal head_idx
            while head_idx < len(head_units) and \
                    head_units[head_idx][0] <= h_done:
                h, kind = head_units[head_idx]
                head_idx += 1
                if kind == 'xn':
                    emit_xn(h)
                elif kind == 'qkkq':
                    emit_qkkq(h)
                else:
                    emit_vt(h)

        # attention stream state
        e2 = {}    # chunk -> fp8 tile (both engines write it)
        ps_a = {}  # chunk -> PSUM accum tile
        av_started = {}

        def alloc_chunk(ic):
            e2[ic] = e2p.tile([P, NST, T], FP8, tag="e2", name="e2")

        def emit_s2_group(ic, g):
            ps_s = pss.tile([P, 2, T], F32, tag="s", name="ps_s")
            for jj in range(2):
                sj = 2 * g + jj
                hb = (sj % 2) * CH
                kv = 1 - (sj % 2)
                qv = sj % 2
                nc.tensor.matmul(ps_s[:, jj, :],
                                 qk2[hb:hb + CH, kv, ts(sj, P)],
                                 qk2[hb:hb + CH, qv, ts(ic, T)],
                                 start=True, stop=True,
                                 tile_position=(hb, 0))
            dst = e2[ic][:, 2 * g:2 * g + 2, :]
            if PATTERN[g] == 'A':
                nc.scalar.activation(dst, ps_s, AF.Exp, bias=bneg)
            else:
                nc.vector.tensor_scalar(dst.bitcast(U8), ps_s, K8, C8,
                                        OP.mult, OP.add)

        def emit_av_group(ic, g):
            """One DoubleRow AV matmul consuming group g's pair for chunk ic."""
            first = not av_started.get(ic, False)
            if first:
                ps_a[ic] = psa.tile([CH + 1, T], F32, tag="a", name="ps_a")
                av_started[ic] = True
            nc.tensor.matmul(ps_a[ic], vt8[:, 2 * g:2 * g + 2, 0:CH + 1],
                             e2[ic][:, 2 * g:2 * g + 2, :],
                             start=first, stop=False,
                             perf_mode=DR, skip_group_check=True)

        def emit_av_finish(ic):
            azt = work.tile([CH + 1, T], BF16, tag="az", name="azt")
            nc.vector.tensor_copy(azt, ps_a[ic])
            nc.sync.dma_start(z_d[0:1, ts(ic, T)], azt[CH:CH + 1, :])
            ysb = work.tile([P, 2, T], BF16, tag="y", name="ysb")
            # mo=0 in the "y" bank, mo=1 in the freed "a" bank: the two yproj
            # matmuls issue back-to-back, copies go to different engines
            ps_y0 = psy.tile([P, T], F32, tag="y", name="ps_y0")
            ps_y1 = psa.tile([P, T], F32, tag="a", name="ps_y1")
            nc.tensor.matmul(ps_y0, wo_sb[:, 0, :], azt[0:CH, :],
                             start=True, stop=True)
            nc.tensor.matmul(ps_y1, wo_sb[:, 1, :], azt[0:CH, :],
                             start=True, stop=True)
            nc.scalar.copy(ysb[:, 0, :], ps_y0)
            nc.vector.tensor_copy(ysb[:, 1, :], ps_y1)
            nc.sync.dma_start(yp_d[:, :, ts(ic, T)], ysb)

        for ic in range(NCHUNK):
            alloc_chunk(ic)
            for g in range(NG):
                if ic == 0:
                    # projection just-in-time for the first group, then one
                    # x-chunk ahead of S2 needs
                    need_h = (2 * g + 1) // 4
                    run_head_until(min(need_h + (1 if g else 0), NCHUNK - 1))
                # AV first: its deps are long satisfied, keeps the in-order PE
                # queue from head-of-line blocking on a not-yet-ready S2 pair
                if g >= LAG:
                    emit_av_group(ic, g - LAG)
                elif ic > 0:
                    gg = NG - LAG + g
                    emit_av_group(ic - 1, gg)
                    if gg == NG - 1:
                        emit_av_finish(ic - 1)
                        del e2[ic - 1], ps_a[ic - 1]
                emit_s2_group(ic, g)
        for gg in range(NG - LAG, NG):
            emit_av_group(NCHUNK - 1, gg)
        emit_av_finish(NCHUNK - 1)

    nc.compile()
    return nc


def make_core_inputs(x, gn_scale, gn_bias, w_qkv, b_qkv, w_out, b_out):
    """Shard full inputs into 8 per-core input maps (batch n, head h)."""
    N, C, D, H, W = x.shape
    l = D * H * W
    xf = np.ascontiguousarray(x.reshape(N, C, l), dtype=np.float32)
    scale = np.float32(1.0 / np.sqrt(np.sqrt(CH)))
    gnsc = np.ascontiguousarray(gn_scale.reshape(2, P).T, dtype=np.float32)
    gnbi = np.ascontiguousarray(gn_bias.reshape(2, P).T, dtype=np.float32)
    in_maps = []
    # chunk-major layout [P, chunk, po, T]
    xbf = [np.ascontiguousarray(
        xf[n].reshape(2, P, NCHUNK, T).transpose(1, 2, 0, 3)).astype(
            ml_dtypes.bfloat16)
        for n in range(N)]
    for core in range(N_CORES):
        n, h = divmod(core, 4)
        xn_ = xbf[n]
        wq_h = w_qkv[h * CH:(h + 1) * CH] * scale
        wk_h = w_qkv[C + h * CH:C + (h + 1) * CH] * scale
        wv_h = w_qkv[2 * C + h * CH:2 * C + (h + 1) * CH]
        rows = np.concatenate([wq_h, wk_h, wv_h, wk_h, wq_h], axis=0)  # [320, 256]
        wq = np.ascontiguousarray(
            rows.T.reshape(2, P, 320).transpose(1, 0, 2), dtype=np.float32)
        bq_h = b_qkv[h * CH:(h + 1) * CH] * scale
        bk_h = b_qkv[C + h * CH:C + (h + 1) * CH] * scale
        bqk = np.stack([np.concatenate([bq_h, bk_h]),
                        np.concatenate([bk_h, bq_h])], axis=1).astype(np.float32)
        bv = np.ascontiguousarray(b_qkv[2 * C + h * CH:2 * C + (h + 1) * CH],
                                  dtype=np.float32)
        wo = np.ascontiguousarray(
            w_out[:, h * CH:(h + 1) * CH].T.reshape(CH, 2, P), dtype=np.float32)
        gm = np.zeros((P, 8), np.float32)
        for g in range(8):
            gm[g * 16:(g + 1) * 16, g] = 1.0
        in_maps.append({
            "xin": xn_, "wqkvT": wq, "bqk": np.ascontiguousarray(bqk),
            "bv": bv, "woutT": wo, "gnsc": gnsc, "gnbi": gnbi, "gmask_in": gm,
        })
    return in_maps


def combine_outputs(results, x, b_out):
    """Host gather: y = sum_h yp/z per batch + b_out + residual."""
    N, C, D, H, W = x.shape
    l = D * H * W
    xf = x.reshape(N, C, l)
    y = np.zeros((N, C, l), np.float32)
    for core, res in enumerate(results):
        n = core // 4
        yp = np.asarray(res["yp"]).astype(np.float32)
        yp = yp.reshape(P, 2, l).transpose(1, 0, 2).reshape(C, l)
        z = np.asarray(res["zout"]).astype(np.float32).reshape(l)
        y[n] += yp / z[None, :]
    y += b_out.astype(np.float32)[None, :, None] + xf
    return y.reshape(N, C, D, H, W).astype(np.float32)


_NC_CACHE = {}


def get_nc():
    if "nc" not in _NC_CACHE:
        _NC_CACHE["nc"] = build_attention_nc()
    return _NC_CACHE["nc"]


def kernel(x, gn_scale, gn_bias, w_qkv, b_qkv, w_out, b_out, _trace=False):
    from concourse.bass_utils import run_bass_kernel_spmd
    x = np.asarray(x); gn_scale = np.asarray(gn_scale); gn_bias = np.asarray(gn_bias)
    w_qkv = np.asarray(w_qkv); b_qkv = np.asarray(b_qkv)
    w_out = np.asarray(w_out); b_out = np.asarray(b_out)
    nc = get_nc()
    in_maps = make_core_inputs(x, gn_scale, gn_bias, w_qkv, b_qkv, w_out, b_out)
    res = run_bass_kernel_spmd(nc, in_maps, core_ids=list(range(N_CORES)),
                               trace=_trace)
    out = combine_outputs(res.results, x, b_out)
    if _trace:
        kernel.last_results = res
    return out


if __name__ == "__main__":
    sys.path.insert(0, os.path.dirname(os.path.abspath(__file__)))
    import reference
    inputs = {k: np.asarray(v) for k, v in reference.setup_inputs().items()}
    expected = np.asarray(reference.reference(**inputs))
    got = kernel(**inputs)
    err = np.abs(got - expected).max()
    rel = err / np.abs(expected).max()
    print("abs err:", err, "rel err:", rel)
